# revision 14
# baseline (speedup 1.0000x reference)
"""DCRNN encoder (T=24, B=64, N=207, IN=2, H=64, K=2, L=2) on 8 TRN2 cores.

Sharding: data-parallel over batch (8 batches/core), weights + supports
replicated. Each core runs the full 24-step scan for its batch shard; no
collectives. Host packs inputs into kernel-friendly layouts and unpacks
outputs.

Device layouts per core (b = 8 local batches):
  fm (feature-major): SBUF [feat, b*207]  col = b*207 + n
  nm (node-major):    SBUF [node_chunk(128|79), ch, j*8+b]  (j = feature)

Per step, per layer: gates = sigmoid(sum_k A_k cat(x,h) Wk + bg),
cand = tanh(sum_k A_k cat(x, r*h) Wk + bc), h' = u*h + (1-u)*c, with
A = [I, S, M2], M2 = 2 S@S - I (host-precomputed). Diffusion matmuls use
the activation as the stationary operand (lhsT = per-batch-pair nm tile)
against rhs [S^T | M2^T], which yields feature-major outputs directly.
"""

import numpy as np
from contextlib import ExitStack

import concourse.bass as bass
import concourse.bacc as bacc
import concourse.tile as tile
from concourse import mybir
from concourse.bass_utils import run_bass_kernel_spmd

T, B, N, IN, H = 24, 64, 207, 2, 64
NCORES = 8
BSH = B // NCORES          # 8 batches per core
F = BSH * N                # 1656 fm columns
NCH = (128, 79)            # node chunks
NSL = 4                    # gemm column slices (one per batch pair)
SLW = 2 * N                # slice width = 414
f32 = mybir.dt.float32
AF = mybir.ActivationFunctionType


# ---------------------------------------------------------------- program ---

def build_program(t_steps=T):
    nc = bacc.Bacc("TRN2", target_bir_lowering=False, debug=False,
                   num_devices=NCORES)
    d = {}

    def din(name, shape):
        d[name] = nc.dram_tensor(name, list(shape), f32, kind="ExternalInput").ap()

    def dout(name, shape):
        d[name] = nc.dram_tensor(name, list(shape), f32, kind="ExternalOutput").ap()

    din("sm2t", (2, 128, SLW))        # [S^T | M2^T] per node chunk, zero-padded
    din("ident", (128, 128))
    din("identhi", (128, 64))
    din("wg0c0", (128, 128)); din("wg0c1", (70, 128))
    din("wc0c0", (128, 64));  din("wc0c1", (70, 64))
    din("wg1", (3, 128, 128)); din("wc1", (3, 128, 64))
    din("bg0", (128, 1)); din("bc0", (64, 1))
    din("bg1", (128, 1)); din("bc1", (64, 1))
    din("x0nm", (t_steps, 2, 128, 16))   # [t, ch, m, c*8+b]
    din("x0fm", (t_steps, 2, F))         # [t, c, b*207+n]
    din("hfm0", (128, F))                # [l*64+j, b*207+n]
    din("hnm0", (2, 128, 1024))          # [ch, m, j*8+b], j = l*64+jj
    dout("cur", (t_steps, 2, 128, 512))  # [t, ch, m, jj*8+b] = h1
    dout("hlast", (2, 128, 1024))        # final [ch, m, (l*64+jj)*8+b]

    with tile.TileContext(nc) as tc, ExitStack() as ctx:
        _emit(ctx, tc, nc, d, t_steps)
    nc.compile()
    return nc


def _emit(ctx, tc, nc, d, t_steps):
    cp = ctx.enter_context(tc.tile_pool(name="consts", bufs=1))
    sp = ctx.enter_context(tc.tile_pool(name="work", bufs=1))
    pp = ctx.enter_context(tc.tile_pool(name="ps", bufs=1, space="PSUM"))
    dma = nc.sync.dma_start

    # round-robin evacuation engine
    _ev = [0]

    def evac(out, in_):
        _ev[0] ^= 1
        if _ev[0]:
            nc.scalar.copy(out, in_)
        else:
            nc.vector.tensor_copy(out, in_)

    # ---- constants
    smt = cp.tile([128, 2, SLW], f32, name="smt")
    for ch in range(2):
        dma(smt[:, ch, :], d["sm2t"][ch])
    ident = cp.tile([128, 128], f32, name="ident")
    dma(ident[:], d["ident"][:])
    identhi = cp.tile([128, 64], f32, name="identhi")
    dma(identhi[:], d["identhi"][:])
    wg0c0 = cp.tile([128, 128], f32, name="wg0c0"); dma(wg0c0[:], d["wg0c0"][:])
    wg0c1 = cp.tile([70, 128], f32, name="wg0c1"); dma(wg0c1[:], d["wg0c1"][:])
    wc0c0 = cp.tile([128, 64], f32, name="wc0c0"); dma(wc0c0[:], d["wc0c0"][:])
    wc0c1 = cp.tile([70, 64], f32, name="wc0c1"); dma(wc0c1[:], d["wc0c1"][:])
    wg1 = cp.tile([128, 3, 128], f32, name="wg1")
    wc1 = cp.tile([128, 3, 64], f32, name="wc1")
    for k in range(3):
        dma(wg1[:, k, :], d["wg1"][k])
        dma(wc1[:, k, :], d["wc1"][k])
    bg0 = cp.tile([128, 1], f32, name="bg0"); dma(bg0[:], d["bg0"][:])
    bc0 = cp.tile([64, 1], f32, name="bc0"); dma(bc0[:], d["bc0"][:])
    bg1 = cp.tile([128, 1], f32, name="bg1"); dma(bg1[:], d["bg1"][:])
    bc1 = cp.tile([64, 1], f32, name="bc1"); dma(bc1[:], d["bc1"][:])

    def wtile(name, shape, bufs, t):
        return sp.tile(shape, f32, tag=name, bufs=bufs, name=f"{name}_{t}")

    def nm_bview(tl, ch):
        # [m, ch, j*8+b] -> [m, b, j]
        return tl[:, ch, :].rearrange("m (j b) -> m b j", b=BSH)

    def nm_jview(tl, ch):
        # [m, ch, j*8+b] -> [m, j, b]
        return tl[:, ch, :].rearrange("m (j b) -> m j b", b=BSH)

    # ---- initial state
    # hx_fm(t) = [h0(t+1) | h1(t)] fm. nm tiles use cols b*64+j so that
    # batch pairs are contiguous 128-col blocks (single-free-dim lhsT APs).
    # L1's x-diffusion at step t == L0's h-diffusion at step t+1: computed
    # once per step as G1 on h0(t+1); h1/z0/z1 get their own pair groups.
    hx_fm = wtile("hx_fm", [128, F], 2, 0)
    dma(hx_fm[64:128, :], d["hfm0"][64:128, :])
    h1fm = wtile("h1fm", [64, F], 2, 0)
    dma(h1fm[:], d["hfm0"][64:128, :])
    h0nm_p = wtile("h0nm", [128, 2, 512], 2, "init")
    h1nm = wtile("h1nm", [128, 2, 512], 2, 0)
    for ch in range(2):
        dma(h0nm_p[0:NCH[ch], ch, :], d["hnm0"][ch, 0:NCH[ch], 0:512])
        dma(h1nm[0:NCH[ch], ch, :], d["hnm0"][ch, 0:NCH[ch], 512:1024])
    l0ch0 = wtile("l0ch0", [128, F], 2, 0)
    dma(l0ch0[0:64, :], d["hfm0"][0:64, :])
    l0ch1 = wtile("l0ch1", [70, F], 2, 0)

    def pair_diff(nm_tile, tag, t):
        """4 psum tiles [128,414]; pair p rows 0:64 = batch 2p's [S.|M2.],
        rows 64:128 = batch 2p+1's."""
        outs = []
        for p in range(NSL):
            ps = pp.tile([128, SLW], f32, tag="mm", bufs=5, name=f"{tag}_{t}_{p}")
            for ch in range(2):
                nc.tensor.matmul(ps[:], nm_tile[0:NCH[ch], ch, 128 * p:128 * (p + 1)],
                                 smt[0:NCH[ch], ch, :], start=(ch == 0),
                                 stop=(ch == 1))
            outs.append(ps)
        return outs

    def diff_evac(outs, dstS, rowS, dstM, rowM):
        for p, ps in enumerate(outs):
            for i in range(2):
                b = 2 * p + i
                cols = slice(b * N, (b + 1) * N)
                evac(dstS[rowS:rowS + 64, cols], ps[64 * i:64 * i + 64, 0:N])
                evac(dstM[rowM:rowM + 64, cols], ps[64 * i:64 * i + 64, N:2 * N])

    def tr_group(src, src_row, dst_nm, idt, tag, t):
        """transpose fm rows [src_row:src_row+64] -> nm tile (cols b*64+j)."""
        for b in range(BSH):
            for ch in range(2):
                pst = pp.tile([128, 64], f32, tag="sm", bufs=2,
                              name=f"{tag}_{t}_{b}_{ch}")
                cols = slice(b * N + 128 * ch, b * N + 128 * ch + NCH[ch])
                nc.tensor.transpose(pst[0:NCH[ch], :],
                                    src[src_row:src_row + 64, cols], idt)
                evac(dst_nm[0:NCH[ch], ch, b * 64:(b + 1) * 64], pst[0:NCH[ch], :])

    # prologue: diffuse h0(0) for step 0's L0 GEMM
    g1p = pair_diff(h0nm_p, "d0p", "init")
    diff_evac(g1p, l0ch0, 64, l0ch1, 0)

    for t in range(t_steps):
        # ---------- tiles
        hx_fm_n = wtile("hx_fm", [128, F], 2, t + 1)
        h1fm_n = wtile("h1fm", [64, F], 2, t + 1)
        l0ch0_n = wtile("l0ch0", [128, F], 2, t + 1)
        l0ch1_n = wtile("l0ch1", [70, F], 2, t + 1)
        c1ch0 = wtile("c1ch0", [128, F], 2, t)
        h0nm = wtile("h0nm", [128, 2, 512], 2, t)
        h1nm_n = wtile("h1nm", [128, 2, 512], 2, t + 1)
        zch1 = wtile("zch1", [70, F], 2, t)

        # ---------- L0 x-part: diffusion of x_t (2 feats, all batches)
        xnm = wtile("xnm", [128, 2, 16], 3, t)
        for ch in range(2):
            dma(xnm[0:NCH[ch], ch, :], d["x0nm"][t, ch, 0:NCH[ch], :])
        dma(l0ch1[64:66, :], d["x0fm"][t])
        dma(zch1[64:66, :], d["x0fm"][t])
        psx = pp.tile([16, SLW], f32, tag="sm", bufs=2, name=f"psx_{t}")
        for ch in range(2):
            nc.tensor.matmul(psx[:], xnm[0:NCH[ch], ch, :], smt[0:NCH[ch], ch, :],
                             start=(ch == 0), stop=(ch == 1))
        xsb = wtile("xsb", [16, SLW], 3, t)
        nc.scalar.copy(xsb[:], psx[:])
        for op in range(2):
            for c in range(2):
                srcx = xsb[c * 8:(c + 1) * 8, op * N:(op + 1) * N]
                row = 66 + 2 * op + c
                for dst in (l0ch1, zch1):
                    dma(dst[row:row + 1, :].rearrange("o (b n) -> o b n", b=BSH),
                        srcx)

        # ---------- L0 gates GEMM + sigmoid
        r0 = wtile("r0", [64, F], 1, t)
        u0 = wtile("u0", [64, F], 1, t)
        for p in range(NSL):
            sl = slice(p * SLW, (p + 1) * SLW)
            ps = pp.tile([128, SLW], f32, tag="mm", bufs=5, name=f"g0_{t}_{p}")
            nc.tensor.matmul(ps[:], wg0c0[:], l0ch0[:, sl], start=True, stop=False)
            nc.tensor.matmul(ps[:], wg0c1[:], l0ch1[:, sl], start=False, stop=True)
            nc.scalar.activation(r0[:, sl], ps[0:64, :], AF.Sigmoid,
                                 bias=bg0[0:64, 0:1])
            nc.scalar.activation(u0[:, sl], ps[64:128, :], AF.Sigmoid,
                                 bias=bg0[64:128, 0:1])

        # ---------- L0 cand: z0 = r0*h0, transpose, diffuse
        zch0 = wtile("zch0", [128, F], 1, t)
        for p in range(NSL):
            sl = slice(p * SLW, (p + 1) * SLW)
            nc.vector.tensor_mul(zch0[0:64, sl], r0[:, sl], l0ch0[0:64, sl])
        z0nm = wtile("z0nm", [128, 2, 512], 1, t)
        tr_group(zch0, 0, z0nm, ident[0:64, 0:64], "tz0", t)
        g3 = pair_diff(z0nm, "dz0", t)
        diff_evac(g3, zch0, 64, zch1, 0)

        # ---------- L0 cand GEMM + tanh, h0' update
        c0 = wtile("c0", [64, F], 1, t)
        tmp0 = sp.tile([64, F], f32, tag="tmp", bufs=1, name=f"tmp0_{t}")
        for p in range(NSL):
            sl = slice(p * SLW, (p + 1) * SLW)
            ps = pp.tile([64, SLW], f32, tag="mm", bufs=5, name=f"c0_{t}_{p}")
            nc.tensor.matmul(ps[:], wc0c0[:], zch0[:, sl], start=True, stop=False)
            nc.tensor.matmul(ps[:], wc0c1[:], zch1[:, sl], start=False, stop=True)
            nc.scalar.activation(c0[:, sl], ps[:], AF.Tanh, bias=bc0[:, 0:1])
            # h0' = u*(h0-c) + c -> feeds L1 of this step and L0 of step t+1
            nc.vector.tensor_sub(tmp0[:, sl], l0ch0[0:64, sl], c0[:, sl])
            nc.vector.tensor_mul(tmp0[:, sl], u0[:, sl], tmp0[:, sl])
            nc.vector.tensor_add(hx_fm[0:64, sl], tmp0[:, sl], c0[:, sl])
            nc.gpsimd.tensor_copy(l0ch0_n[0:64, sl], hx_fm[0:64, sl])
            nc.gpsimd.tensor_copy(c1ch0[0:64, sl], hx_fm[0:64, sl])

        # ---------- h0(t+1) -> nm, then G1 diffusion (shared L1-x / next L0-h)
        tr_group(hx_fm, 0, h0nm, ident[0:64, 0:64], "th0", t)
        sxh1 = wtile("sxh1", [128, F], 2, t)
        m2xh1 = wtile("m2xh1", [128, F], 2, t)
        g1 = pair_diff(h0nm, "d1x", t)
        diff_evac(g1, sxh1, 0, m2xh1, 0)
        zf1a = wtile("zf1a", [128, F], 1, t)
        zf1b = wtile("zf1b", [128, F], 1, t)
        nc.gpsimd.tensor_copy(l0ch0_n[64:128, :], sxh1[0:64, :])
        nc.gpsimd.tensor_copy(l0ch1_n[0:64, :], m2xh1[0:64, :])
        nc.gpsimd.tensor_copy(zf1a[0:64, :], sxh1[0:64, :])
        nc.gpsimd.tensor_copy(zf1b[0:64, :], m2xh1[0:64, :])

        # ---------- L1 gates: h1 diffusion + GEMM + sigmoid
        g2 = pair_diff(h1nm, "d1h", t)
        diff_evac(g2, sxh1, 64, m2xh1, 64)
        r1 = wtile("r1", [64, F], 1, t)
        u1 = wtile("u1", [64, F], 1, t)
        for p in range(NSL):
            sl = slice(p * SLW, (p + 1) * SLW)
            ps = pp.tile([128, SLW], f32, tag="mm", bufs=5, name=f"g1_{t}_{p}")
            nc.tensor.matmul(ps[:], wg1[:, 0, :], hx_fm[:, sl], start=True, stop=False)
            nc.tensor.matmul(ps[:], wg1[:, 1, :], sxh1[:, sl], start=False, stop=False)
            nc.tensor.matmul(ps[:], wg1[:, 2, :], m2xh1[:, sl], start=False, stop=True)
            nc.scalar.activation(r1[:, sl], ps[0:64, :], AF.Sigmoid,
                                 bias=bg1[0:64, 0:1])
            nc.scalar.activation(u1[:, sl], ps[64:128, :], AF.Sigmoid,
                                 bias=bg1[64:128, 0:1])

        # ---------- L1 cand: z1 = r1*h1, transpose, diffuse
        for p in range(NSL):
            sl = slice(p * SLW, (p + 1) * SLW)
            nc.vector.tensor_mul(c1ch0[64:128, sl], r1[:, sl], h1fm[:, sl])
        z1nm = wtile("z1nm", [128, 2, 512], 1, t)
        tr_group(c1ch0, 64, z1nm, identhi[64:128, :], "tz1", t)
        g4 = pair_diff(z1nm, "dz1", t)
        diff_evac(g4, zf1a, 64, zf1b, 64)

        # ---------- L1 cand GEMM + tanh, h1' update
        c1 = wtile("c1", [64, F], 1, t)
        tmp1 = sp.tile([64, F], f32, tag="tmp", bufs=1, name=f"tmp1_{t}")
        for p in range(NSL):
            sl = slice(p * SLW, (p + 1) * SLW)
            ps = pp.tile([64, SLW], f32, tag="mm", bufs=5, name=f"cc1_{t}_{p}")
            nc.tensor.matmul(ps[:], wc1[:, 0, :], c1ch0[:, sl], start=True, stop=False)
            nc.tensor.matmul(ps[:], wc1[:, 1, :], zf1a[:, sl], start=False, stop=False)
            nc.tensor.matmul(ps[:], wc1[:, 2, :], zf1b[:, sl], start=False, stop=True)
            nc.scalar.activation(c1[:, sl], ps[:], AF.Tanh, bias=bc1[:, 0:1])
            nc.vector.tensor_sub(tmp1[:, sl], h1fm[:, sl], c1[:, sl])
            nc.vector.tensor_mul(tmp1[:, sl], u1[:, sl], tmp1[:, sl])
            nc.vector.tensor_add(hx_fm_n[64:128, sl], tmp1[:, sl], c1[:, sl])
            nc.gpsimd.tensor_copy(h1fm_n[:, sl], hx_fm_n[64:128, sl])

        # ---------- h1(t+1) -> nm, outputs
        tr_group(hx_fm_n, 64, h1nm_n, identhi[64:128, :], "th1", t)
        dma(d["cur"][t, 0], h1nm_n[:, 0, :])
        dma(d["cur"][t, 1, 0:79], h1nm_n[0:79, 1, :])
        if t == t_steps - 1:
            dma(d["hlast"][0, :, 0:512], h0nm[:, 0, :])
            dma(d["hlast"][1, 0:79, 0:512], h0nm[0:79, 1, :])
            dma(d["hlast"][0, :, 512:1024], h1nm_n[:, 0, :])
            dma(d["hlast"][1, 0:79, 512:1024], h1nm_n[0:79, 1, :])

        hx_fm, l0ch0, l0ch1, h1nm = hx_fm_n, l0ch0_n, l0ch1_n, h1nm_n
        h1fm = h1fm_n


# ------------------------------------------------------------- host pack ---

def prep_inputs(inputs, initial_hidden_state, supports,
                w_gate0, b_gate0, w_cand0, b_cand0,
                w_gate1, b_gate1, w_cand1, b_cand1, t_steps=T):
    S = np.asarray(supports[0], np.float32)
    M2 = (2.0 * S @ S - np.eye(N, dtype=np.float32)).astype(np.float32)
    sm2t = np.zeros((2, 128, SLW), np.float32)
    for ch in range(2):
        m0, msz = 128 * ch, NCH[ch]
        sm2t[ch, 0:msz, 0:N] = S.T[m0:m0 + msz]
        sm2t[ch, 0:msz, N:2 * N] = M2.T[m0:m0 + msz]

    def l0_rows(w):
        blk = [w[66 * k:66 * (k + 1)] for k in range(3)]
        c0 = np.concatenate([blk[0][2:66], blk[1][2:66]], 0)
        c1 = np.concatenate([blk[2][2:66], blk[0][0:2], blk[1][0:2], blk[2][0:2]], 0)
        return np.ascontiguousarray(c0), np.ascontiguousarray(c1)

    wg0c0, wg0c1 = l0_rows(np.asarray(w_gate0, np.float32))
    wc0c0, wc0c1 = l0_rows(np.asarray(w_cand0, np.float32))
    wg1 = np.asarray(w_gate1, np.float32).reshape(3, 128, 128)
    wc1 = np.asarray(w_cand1, np.float32).reshape(3, 128, 64)

    x = np.asarray(inputs, np.float32)[:t_steps]          # [t, B, N, IN]
    h0 = np.asarray(initial_hidden_state, np.float32)     # [L, B, N*H]

    shared = {
        "sm2t": sm2t, "ident": np.eye(128, dtype=np.float32),
        "identhi": np.vstack([np.zeros((64, 64), np.float32),
                              np.eye(64, dtype=np.float32)]),
        "wg0c0": wg0c0, "wg0c1": wg0c1, "wc0c0": wc0c0, "wc0c1": wc0c1,
        "wg1": wg1, "wc1": wc1,
        "bg0": np.asarray(b_gate0, np.float32).reshape(128, 1),
        "bc0": np.asarray(b_cand0, np.float32).reshape(64, 1),
        "bg1": np.asarray(b_gate1, np.float32).reshape(128, 1),
        "bc1": np.asarray(b_cand1, np.float32).reshape(64, 1),
    }

    in_maps = []
    for c in range(NCORES):
        bs = slice(c * BSH, (c + 1) * BSH)
        xc = x[:, bs]                                     # [t, 8, N, 2]
        x0nm = np.zeros((t_steps, 2, 128, 16), np.float32)
        for ch in range(2):
            m0, msz = 128 * ch, NCH[ch]
            # [t, msz, c, b] <- [t, b, m, c]
            x0nm[:, ch, 0:msz] = xc[:, :, m0:m0 + msz, :].transpose(
                0, 2, 3, 1).reshape(t_steps, msz, 16)
        x0fm = np.ascontiguousarray(
            xc.transpose(0, 3, 1, 2).reshape(t_steps, 2, F))
        hc = h0[:, bs].reshape(2, BSH, N, H)              # [l, b, n, j]
        hfm0 = np.ascontiguousarray(
            hc.transpose(0, 3, 1, 2).reshape(128, F))
        hnm0 = np.zeros((2, 128, 1024), np.float32)
        for ch in range(2):
            m0, msz = 128 * ch, NCH[ch]
            # [m, l, b, j] <- [l, b, m, j]
            hnm0[ch, 0:msz] = hc[:, :, m0:m0 + msz, :].transpose(
                2, 0, 1, 3).reshape(msz, 1024)
        in_maps.append(dict(shared, x0nm=x0nm, x0fm=x0fm, hfm0=hfm0, hnm0=hnm0))
    return in_maps


def post_outputs(results, t_steps=T):
    cur = np.empty((t_steps, B, N, H), np.float32)
    hid = np.empty((2, B, N, H), np.float32)
    for c, r in enumerate(results):
        bs = slice(c * BSH, (c + 1) * BSH)
        cc = r["cur"].reshape(t_steps, 2 * 128, BSH, H)   # [t, n, b, j]
        cur[:, bs] = cc[:, 0:N].transpose(0, 2, 1, 3)
        hl = r["hlast"].reshape(2 * 128, 2, BSH, H)       # [n, l, b, j]
        hid[:, bs] = hl[0:N].transpose(1, 2, 0, 3)
    return (hid.reshape(2, B, N * H),
            cur.reshape(t_steps, B, N * H))


_CACHE = {}


def kernel(**inputs):
    nc = _CACHE.get("nc")
    if nc is None:
        nc = build_program(T)
        _CACHE["nc"] = nc
    in_maps = prep_inputs(**inputs)
    res = run_bass_kernel_spmd(nc, in_maps, list(range(NCORES)))
    return post_outputs(res.results)


# revision 16
# speedup vs baseline: 1.5427x; 1.5427x over previous
"""DCRNN encoder (T=24, B=64, N=207, IN=2, H=64, K=2, L=2) on 8 TRN2 cores.

Sharding: data-parallel over batch (8 batches/core), weights + supports
replicated. Each core runs the full 24-step scan for its batch shard; no
collectives. Host packs inputs into kernel-friendly layouts and unpacks
outputs.

Device layouts per core (b = 8 local batches):
  fm (feature-major): SBUF [feat, b*207]  col = b*207 + n
  nm (node-major):    SBUF [node_chunk(128|79), ch, j*8+b]  (j = feature)

Per step, per layer: gates = sigmoid(sum_k A_k cat(x,h) Wk + bg),
cand = tanh(sum_k A_k cat(x, r*h) Wk + bc), h' = u*h + (1-u)*c, with
A = [I, S, M2], M2 = 2 S@S - I (host-precomputed). Diffusion matmuls use
the activation as the stationary operand (lhsT = per-batch-pair nm tile)
against rhs [S^T | M2^T], which yields feature-major outputs directly.
"""

import numpy as np
from contextlib import ExitStack

import concourse.bass as bass
import concourse.bacc as bacc
import concourse.tile as tile
from concourse import mybir
from concourse.bass_utils import run_bass_kernel_spmd

T, B, N, IN, H = 24, 64, 207, 2, 64
NCORES = 8
BSH = B // NCORES          # 8 batches per core
F = BSH * N                # 1656 fm columns
NCH = (128, 79)            # node chunks
NSL = 4                    # gemm column slices (one per batch pair)
SLW = 2 * N                # slice width = 414
f32 = mybir.dt.float32
f32r = mybir.dt.float32r
AF = mybir.ActivationFunctionType


# ---------------------------------------------------------------- program ---

def build_program(t_steps=T):
    nc = bacc.Bacc("TRN2", target_bir_lowering=False, debug=False,
                   num_devices=NCORES)
    d = {}

    def din(name, shape, dt=f32r):
        d[name] = nc.dram_tensor(name, list(shape), dt, kind="ExternalInput").ap()

    def dout(name, shape, dt=f32r):
        d[name] = nc.dram_tensor(name, list(shape), dt, kind="ExternalOutput").ap()

    din("sm2t", (2, 128, SLW))        # [S^T | M2^T] per node chunk, zero-padded
    din("ident", (128, 128))
    din("identhi", (128, 64))
    din("wg0c0", (128, 128)); din("wg0c1", (70, 128))
    din("wc0c0", (128, 64));  din("wc0c1", (70, 64))
    din("wg1", (3, 128, 128)); din("wc1", (3, 128, 64))
    din("bg0", (128, 1), f32); din("bc0", (64, 1), f32)
    din("bg1", (128, 1), f32); din("bc1", (64, 1), f32)
    din("x0f6", (t_steps, 6, F))         # [t, (x|Sx|M2x)(c), b*207+n]
    din("hfm0", (128, F))                # [l*64+j, b*207+n]
    din("hnm0", (2, 128, 1024))          # [ch, m, j*8+b], j = l*64+jj
    dout("cur", (t_steps, 2, 128, 512))  # [t, ch, m, jj*8+b] = h1
    dout("hlast", (2, 128, 1024))        # final [ch, m, (l*64+jj)*8+b]

    with tile.TileContext(nc) as tc, ExitStack() as ctx:
        _emit(ctx, tc, nc, d, t_steps)
    nc.compile()
    return nc


def _emit(ctx, tc, nc, d, t_steps):
    cp = ctx.enter_context(tc.tile_pool(name="consts", bufs=1))
    sp = ctx.enter_context(tc.tile_pool(name="work", bufs=1))
    pp = ctx.enter_context(tc.tile_pool(name="ps", bufs=1, space="PSUM"))
    dma = nc.sync.dma_start

    # round-robin evacuation engine
    _ev = [0]

    def evac(out, in_):
        _ev[0] ^= 1
        if _ev[0]:
            nc.scalar.copy(out, in_)
        else:
            nc.vector.tensor_copy(out, in_)

    # ---- constants
    smt = cp.tile([128, 2, SLW], f32r, name="smt")
    for ch in range(2):
        dma(smt[:, ch, :], d["sm2t"][ch])
    ident = cp.tile([128, 128], f32r, name="ident")
    dma(ident[:], d["ident"][:])
    identhi = cp.tile([128, 64], f32r, name="identhi")
    dma(identhi[:], d["identhi"][:])
    wg0c0 = cp.tile([128, 128], f32r, name="wg0c0"); dma(wg0c0[:], d["wg0c0"][:])
    wg0c1 = cp.tile([70, 128], f32r, name="wg0c1"); dma(wg0c1[:], d["wg0c1"][:])
    wc0c0 = cp.tile([128, 64], f32r, name="wc0c0"); dma(wc0c0[:], d["wc0c0"][:])
    wc0c1 = cp.tile([70, 64], f32r, name="wc0c1"); dma(wc0c1[:], d["wc0c1"][:])
    wg1 = cp.tile([128, 3, 128], f32r, name="wg1")
    wc1 = cp.tile([128, 3, 64], f32r, name="wc1")
    for k in range(3):
        dma(wg1[:, k, :], d["wg1"][k])
        dma(wc1[:, k, :], d["wc1"][k])
    bg0 = cp.tile([128, 1], f32, name="bg0"); dma(bg0[:], d["bg0"][:])
    bc0 = cp.tile([64, 1], f32, name="bc0"); dma(bc0[:], d["bc0"][:])
    bg1 = cp.tile([128, 1], f32, name="bg1"); dma(bg1[:], d["bg1"][:])
    bc1 = cp.tile([64, 1], f32, name="bc1"); dma(bc1[:], d["bc1"][:])

    def wtile(name, shape, bufs, t, dt=None):
        return sp.tile(shape, dt or f32r, tag=name, bufs=bufs, name=f"{name}_{t}")

    def nm_bview(tl, ch):
        # [m, ch, j*8+b] -> [m, b, j]
        return tl[:, ch, :].rearrange("m (j b) -> m b j", b=BSH)

    def nm_jview(tl, ch):
        # [m, ch, j*8+b] -> [m, j, b]
        return tl[:, ch, :].rearrange("m (j b) -> m j b", b=BSH)

    # ---- initial state
    # hx_fm(t) = [h0(t+1) | h1(t)] fm. nm tiles use cols b*64+j so that
    # batch pairs are contiguous 128-col blocks (single-free-dim lhsT APs).
    # L1's x-diffusion at step t == L0's h-diffusion at step t+1: computed
    # once per step as G1 on h0(t+1); h1/z0/z1 get their own pair groups.
    hx_fm = wtile("hx_fm", [128, F], 2, 0)
    dma(hx_fm[64:128, :], d["hfm0"][64:128, :])
    h1fm = wtile("h1fm", [64, F], 2, 0, f32)
    dma(h1fm[:], d["hfm0"][64:128, :].bitcast(f32))
    h0nm_p = wtile("h0nm", [128, 2, 512], 2, "init")
    h1nm = wtile("h1nm", [128, 2, 512], 2, 0)
    for ch in range(2):
        dma(h0nm_p[0:NCH[ch], ch, :], d["hnm0"][ch, 0:NCH[ch], 0:512])
        dma(h1nm[0:NCH[ch], ch, :], d["hnm0"][ch, 0:NCH[ch], 512:1024])
    l0ch0 = wtile("l0ch0", [128, F], 2, 0)
    dma(l0ch0[0:64, :], d["hfm0"][0:64, :])
    l0ch1 = wtile("l0ch1", [70, F], 2, 0)

    def pair_diff(nm_tile, tag, t):
        """4 psum tiles [128,414]; pair p rows 0:64 = batch 2p's [S.|M2.],
        rows 64:128 = batch 2p+1's."""
        outs = []
        for p in range(NSL):
            ps = pp.tile([128, SLW], f32, tag="mm", bufs=5, name=f"{tag}_{t}_{p}")
            for ch in range(2):
                nc.tensor.matmul(ps[:], (nm_tile[0:NCH[ch], ch, 128 * p:128 * (p + 1)]),
                                 (smt[0:NCH[ch], ch, :]), start=(ch == 0),
                                 stop=(ch == 1))
            outs.append(ps)
        return outs

    def diff_evac(outs, dstS, rowS, dstM, rowM):
        for p, ps in enumerate(outs):
            for i in range(2):
                b = 2 * p + i
                cols = slice(b * N, (b + 1) * N)
                evac(dstS[rowS:rowS + 64, cols], ps[64 * i:64 * i + 64, 0:N])
                evac(dstM[rowM:rowM + 64, cols], ps[64 * i:64 * i + 64, N:2 * N])

    def tr_group(src, src_row, dst_nm, idt, tag, t):
        """transpose fm rows [src_row:src_row+64] -> nm tile (cols b*64+j)."""
        for b in range(BSH):
            for ch in range(2):
                pst = pp.tile([128, 64], f32r, tag="sm", bufs=2,
                              name=f"{tag}_{t}_{b}_{ch}")
                cols = slice(b * N + 128 * ch, b * N + 128 * ch + NCH[ch])
                nc.tensor.transpose(pst[0:NCH[ch], :],
                                    src[src_row:src_row + 64, cols], idt)
                evac(dst_nm[0:NCH[ch], ch, b * 64:(b + 1) * 64], pst[0:NCH[ch], :])

    # prologue: diffuse h0(0) for step 0's L0 GEMM
    g1p = pair_diff(h0nm_p, "d0p", "init")
    diff_evac(g1p, l0ch0, 64, l0ch1, 0)

    for t in range(t_steps):
        # ---------- tiles
        hx_fm_n = wtile("hx_fm", [128, F], 2, t + 1)
        h1fm_n = wtile("h1fm", [64, F], 2, t + 1, f32)
        l0ch0_n = wtile("l0ch0", [128, F], 2, t + 1)
        l0ch1_n = wtile("l0ch1", [70, F], 2, t + 1)
        c1ch0 = wtile("c1ch0", [128, F], 2, t)
        h0nm = wtile("h0nm", [128, 2, 512], 2, t)
        h1nm_n = wtile("h1nm", [128, 2, 512], 2, t + 1)
        zch1 = wtile("zch1", [70, F], 2, t)

        # ---------- L0 x-part: host-precomputed [x | Sx | M2x] (6 feats)
        dma(l0ch1[64:70, :], d["x0f6"][t])
        dma(zch1[64:70, :], d["x0f6"][t])

        # ---------- L0 gates GEMM + sigmoid
        r0 = wtile("r0", [64, F], 1, t, f32)
        u0 = wtile("u0", [64, F], 1, t, f32)
        for p in range(NSL):
            sl = slice(p * SLW, (p + 1) * SLW)
            ps = pp.tile([128, SLW], f32, tag="mm", bufs=5, name=f"g0_{t}_{p}")
            nc.tensor.matmul(ps[:], (wg0c0[:]), (l0ch0[:, sl]), start=True, stop=False)
            nc.tensor.matmul(ps[:], (wg0c1[:]), (l0ch1[:, sl]), start=False, stop=True)
            nc.scalar.activation(r0[:, sl], ps[0:64, :], AF.Sigmoid,
                                 bias=bg0[0:64, 0:1])
            nc.scalar.activation(u0[:, sl], ps[64:128, :], AF.Sigmoid,
                                 bias=bg0[64:128, 0:1])

        # ---------- L0 cand: z0 = r0*h0, transpose, diffuse
        zch0 = wtile("zch0", [128, F], 1, t)
        for p in range(NSL):
            sl = slice(p * SLW, (p + 1) * SLW)
            nc.vector.tensor_mul(zch0[0:64, sl], r0[:, sl], l0ch0[0:64, sl])
        z0nm = wtile("z0nm", [128, 2, 512], 1, t)
        tr_group(zch0, 0, z0nm, ident[0:64, 0:64], "tz0", t)
        g3 = pair_diff(z0nm, "dz0", t)
        diff_evac(g3, zch0, 64, zch1, 0)

        # ---------- L0 cand GEMM + tanh, h0' update
        c0 = wtile("c0", [64, F], 1, t, f32)
        tmp0 = sp.tile([64, F], f32, tag="tmp", bufs=1, name=f"tmp0_{t}")
        for p in range(NSL):
            sl = slice(p * SLW, (p + 1) * SLW)
            ps = pp.tile([64, SLW], f32, tag="mm", bufs=5, name=f"c0_{t}_{p}")
            nc.tensor.matmul(ps[:], (wc0c0[:]), (zch0[:, sl]), start=True, stop=False)
            nc.tensor.matmul(ps[:], (wc0c1[:]), (zch1[:, sl]), start=False, stop=True)
            nc.scalar.activation(c0[:, sl], ps[:], AF.Tanh, bias=bc0[:, 0:1])
            # h0' = u*(h0-c) + c -> feeds L1 of this step and L0 of step t+1
            nc.vector.tensor_sub(tmp0[:, sl], l0ch0[0:64, sl], c0[:, sl])
            nc.vector.tensor_mul(tmp0[:, sl], u0[:, sl], tmp0[:, sl])
            nc.vector.tensor_add(hx_fm[0:64, sl], tmp0[:, sl], c0[:, sl])
            nc.gpsimd.tensor_copy(l0ch0_n[0:64, sl], hx_fm[0:64, sl])
            nc.gpsimd.tensor_copy(c1ch0[0:64, sl], hx_fm[0:64, sl])

        # ---------- h0(t+1) -> nm, then G1 diffusion (shared L1-x / next L0-h)
        tr_group(hx_fm, 0, h0nm, ident[0:64, 0:64], "th0", t)
        sxh1 = wtile("sxh1", [128, F], 2, t)
        m2xh1 = wtile("m2xh1", [128, F], 2, t)
        g1 = pair_diff(h0nm, "d1x", t)
        diff_evac(g1, sxh1, 0, m2xh1, 0)
        zf1a = wtile("zf1a", [128, F], 1, t)
        zf1b = wtile("zf1b", [128, F], 1, t)
        nc.gpsimd.tensor_copy(l0ch0_n[64:128, :], sxh1[0:64, :])
        nc.gpsimd.tensor_copy(l0ch1_n[0:64, :], m2xh1[0:64, :])
        nc.gpsimd.tensor_copy(zf1a[0:64, :], sxh1[0:64, :])
        nc.gpsimd.tensor_copy(zf1b[0:64, :], m2xh1[0:64, :])

        # ---------- L1 gates: h1 diffusion + GEMM + sigmoid
        g2 = pair_diff(h1nm, "d1h", t)
        diff_evac(g2, sxh1, 64, m2xh1, 64)
        r1 = wtile("r1", [64, F], 1, t)
        u1 = wtile("u1", [64, F], 1, t)
        for p in range(NSL):
            sl = slice(p * SLW, (p + 1) * SLW)
            ps = pp.tile([128, SLW], f32, tag="mm", bufs=5, name=f"g1_{t}_{p}")
            nc.tensor.matmul(ps[:], (wg1[:, 0, :]), (hx_fm[:, sl]), start=True, stop=False)
            nc.tensor.matmul(ps[:], (wg1[:, 1, :]), (sxh1[:, sl]), start=False, stop=False)
            nc.tensor.matmul(ps[:], (wg1[:, 2, :]), (m2xh1[:, sl]), start=False, stop=True)
            nc.scalar.activation(r1[:, sl], ps[0:64, :], AF.Sigmoid,
                                 bias=bg1[0:64, 0:1])
            nc.scalar.activation(u1[:, sl], ps[64:128, :], AF.Sigmoid,
                                 bias=bg1[64:128, 0:1])

        # ---------- L1 cand: z1 = r1*h1, transpose, diffuse
        for p in range(NSL):
            sl = slice(p * SLW, (p + 1) * SLW)
            nc.vector.tensor_mul(c1ch0[64:128, sl], r1[:, sl], h1fm[:, sl])
        z1nm = wtile("z1nm", [128, 2, 512], 1, t)
        tr_group(c1ch0, 64, z1nm, identhi[64:128, :], "tz1", t)
        g4 = pair_diff(z1nm, "dz1", t)
        diff_evac(g4, zf1a, 64, zf1b, 64)

        # ---------- L1 cand GEMM + tanh, h1' update
        c1 = wtile("c1", [64, F], 1, t, f32)
        tmp1 = sp.tile([64, F], f32, tag="tmp", bufs=1, name=f"tmp1_{t}")
        for p in range(NSL):
            sl = slice(p * SLW, (p + 1) * SLW)
            ps = pp.tile([64, SLW], f32, tag="mm", bufs=5, name=f"cc1_{t}_{p}")
            nc.tensor.matmul(ps[:], (wc1[:, 0, :]), (c1ch0[:, sl]), start=True, stop=False)
            nc.tensor.matmul(ps[:], (wc1[:, 1, :]), (zf1a[:, sl]), start=False, stop=False)
            nc.tensor.matmul(ps[:], (wc1[:, 2, :]), (zf1b[:, sl]), start=False, stop=True)
            nc.scalar.activation(c1[:, sl], ps[:], AF.Tanh, bias=bc1[:, 0:1])
            nc.vector.tensor_sub(tmp1[:, sl], h1fm[:, sl], c1[:, sl])
            nc.vector.tensor_mul(tmp1[:, sl], u1[:, sl], tmp1[:, sl])
            nc.vector.tensor_add(hx_fm_n[64:128, sl], tmp1[:, sl], c1[:, sl])
            nc.gpsimd.tensor_copy(h1fm_n[:, sl], hx_fm_n[64:128, sl])

        # ---------- h1(t+1) -> nm, outputs
        tr_group(hx_fm_n, 64, h1nm_n, identhi[64:128, :], "th1", t)
        dma(d["cur"][t, 0], h1nm_n[:, 0, :])
        dma(d["cur"][t, 1, 0:79], h1nm_n[0:79, 1, :])
        if t == t_steps - 1:
            dma(d["hlast"][0, :, 0:512], h0nm[:, 0, :])
            dma(d["hlast"][1, 0:79, 0:512], h0nm[0:79, 1, :])
            dma(d["hlast"][0, :, 512:1024], h1nm_n[:, 0, :])
            dma(d["hlast"][1, 0:79, 512:1024], h1nm_n[0:79, 1, :])

        hx_fm, l0ch0, l0ch1, h1nm = hx_fm_n, l0ch0_n, l0ch1_n, h1nm_n
        h1fm = h1fm_n


# ------------------------------------------------------------- host pack ---

def prep_inputs(inputs, initial_hidden_state, supports,
                w_gate0, b_gate0, w_cand0, b_cand0,
                w_gate1, b_gate1, w_cand1, b_cand1, t_steps=T):
    S = np.asarray(supports[0], np.float32)
    M2 = (2.0 * S @ S - np.eye(N, dtype=np.float32)).astype(np.float32)
    sm2t = np.zeros((2, 128, SLW), np.float32)
    for ch in range(2):
        m0, msz = 128 * ch, NCH[ch]
        sm2t[ch, 0:msz, 0:N] = S.T[m0:m0 + msz]
        sm2t[ch, 0:msz, N:2 * N] = M2.T[m0:m0 + msz]

    def l0_rows(w):
        blk = [w[66 * k:66 * (k + 1)] for k in range(3)]
        c0 = np.concatenate([blk[0][2:66], blk[1][2:66]], 0)
        c1 = np.concatenate([blk[2][2:66], blk[0][0:2], blk[1][0:2], blk[2][0:2]], 0)
        return np.ascontiguousarray(c0), np.ascontiguousarray(c1)

    wg0c0, wg0c1 = l0_rows(np.asarray(w_gate0, np.float32))
    wc0c0, wc0c1 = l0_rows(np.asarray(w_cand0, np.float32))
    wg1 = np.asarray(w_gate1, np.float32).reshape(3, 128, 128)
    wc1 = np.asarray(w_cand1, np.float32).reshape(3, 128, 64)

    x = np.asarray(inputs, np.float32)[:t_steps]          # [t, B, N, IN]
    xf = x.transpose(2, 0, 1, 3).reshape(N, -1)           # [n, t*B*c]
    sx = (S @ xf).reshape(N, t_steps, B, IN).transpose(1, 2, 0, 3)
    m2x = (M2 @ xf).reshape(N, t_steps, B, IN).transpose(1, 2, 0, 3)
    h0 = np.asarray(initial_hidden_state, np.float32)     # [L, B, N*H]

    shared = {
        "sm2t": sm2t, "ident": np.eye(128, dtype=np.float32),
        "identhi": np.vstack([np.zeros((64, 64), np.float32),
                              np.eye(64, dtype=np.float32)]),
        "wg0c0": wg0c0, "wg0c1": wg0c1, "wc0c0": wc0c0, "wc0c1": wc0c1,
        "wg1": wg1, "wc1": wc1,
        "bg0": np.asarray(b_gate0, np.float32).reshape(128, 1),
        "bc0": np.asarray(b_cand0, np.float32).reshape(64, 1),
        "bg1": np.asarray(b_gate1, np.float32).reshape(128, 1),
        "bc1": np.asarray(b_cand1, np.float32).reshape(64, 1),
    }

    in_maps = []
    for c in range(NCORES):
        bs = slice(c * BSH, (c + 1) * BSH)
        xc = x[:, bs]                                     # [t, 8, N, 2]
        x0f6 = np.empty((t_steps, 6, F), np.float32)
        x0f6[:, 0:2] = xc.transpose(0, 3, 1, 2).reshape(t_steps, 2, F)
        x0f6[:, 2:4] = sx[:, bs].transpose(0, 3, 1, 2).reshape(t_steps, 2, F)
        x0f6[:, 4:6] = m2x[:, bs].transpose(0, 3, 1, 2).reshape(t_steps, 2, F)
        hc = h0[:, bs].reshape(2, BSH, N, H)              # [l, b, n, j]
        hfm0 = np.ascontiguousarray(
            hc.transpose(0, 3, 1, 2).reshape(128, F))
        hnm0 = np.zeros((2, 128, 1024), np.float32)
        for ch in range(2):
            m0, msz = 128 * ch, NCH[ch]
            # [m, l, b, j] <- [l, b, m, j]
            hnm0[ch, 0:msz] = hc[:, :, m0:m0 + msz, :].transpose(
                2, 0, 1, 3).reshape(msz, 1024)
        in_maps.append(dict(shared, x0f6=x0f6, hfm0=hfm0, hnm0=hnm0))
    return in_maps


def post_outputs(results, t_steps=T):
    cur = np.empty((t_steps, B, N, H), np.float32)
    hid = np.empty((2, B, N, H), np.float32)
    for c, r in enumerate(results):
        bs = slice(c * BSH, (c + 1) * BSH)
        cc = r["cur"].reshape(t_steps, 2 * 128, BSH, H)   # [t, n, b, j]
        cur[:, bs] = cc[:, 0:N].transpose(0, 2, 1, 3)
        hl = r["hlast"].reshape(2 * 128, 2, BSH, H)       # [n, l, b, j]
        hid[:, bs] = hl[0:N].transpose(1, 2, 0, 3)
    return (hid.reshape(2, B, N * H),
            cur.reshape(t_steps, B, N * H))


_CACHE = {}


def kernel(**inputs):
    nc = _CACHE.get("nc")
    if nc is None:
        nc = build_program(T)
        _CACHE["nc"] = nc
    in_maps = prep_inputs(**inputs)
    res = run_bass_kernel_spmd(nc, in_maps, list(range(NCORES)))
    return post_outputs(res.results)
